# revision 1
# baseline (speedup 1.0000x reference)
"""CenterLoss kernel for Trainium2, data-parallel across 8 NeuronCores.

Math: the reference masks the full [B, C] squared-distance matrix with
one_hot(labels) and clamps to [1e-12, 1e12] before summing.  The mask keeps
only distmat[i, labels[i]]; every other entry becomes clip(0) = 1e-12.  The
kept entries are ~1024 (chi-square-like, 512 dof), so the clamp never binds
on them and the loss reduces to

    loss = ( sum_i ||x_i - c_{l_i}||^2 + B*(C-1)*1e-12 ) / B

Per core (B/8 = 2048 rows), raw bass pipeline, p-major row layout
(shard row 16*p + r lives at partition p, chunk r; r in [0,16)):
  sync   : 4x 1MB x loads (HWDGE, 8KB/partition contiguous descriptors)
  gpsimd : label load + 16x indirect_dma_start (row r: out[p,:] =
           centers[labels[16p+r], :])
  vector : diff = x - g per chunk            [128, 512]
  scalar : Square activation + row-accumulate -> acc[:, r], then acc store
Host sums the 8 x [128, 16] partials in f64 and adds the clamp constant.
"""

import sys
from contextlib import ExitStack

import numpy as np

try:
    import concourse.bass  # noqa: F401
except ImportError:
    sys.path.insert(0, "/opt/trn_rl_repo")

import concourse.bass as bass
import concourse.mybir as mybir
from concourse.bacc import Bacc
from concourse.bass_utils import run_bass_kernel_spmd

B, C, D = 16384, 1000, 512
N_CORES = 8
B_SHARD = B // N_CORES  # 2048
P = 128
NCHUNK = B_SHARD // P  # 16 chunks, chunk r = rows {16p + r}
NXD = 4  # x loads (4 chunks each)
CLAMP_MIN = 1e-12
CLAMP_MAX = 1e12

_NC_CACHE = {}


def build_nc():
    nc = Bacc()
    f32 = mybir.dt.float32
    x_d = nc.declare_dram_parameter("x", [B_SHARD, D], f32, isOutput=False)
    lbl_d = nc.declare_dram_parameter(
        "labels", [P, NCHUNK], mybir.dt.int32, isOutput=False
    )
    cen_d = nc.declare_dram_parameter("centers", [C, D], f32, isOutput=False)
    out_d = nc.declare_dram_parameter("out", [P, NCHUNK], f32, isOutput=True)

    x_r = x_d.rearrange("(p r) d -> p r d", p=P)  # [128, 16, 512], contiguous per p

    with ExitStack() as ctx:
        x_sb = ctx.enter_context(nc.sbuf_tensor("x_sb", [P, NCHUNK, D], f32))
        g_sb = ctx.enter_context(nc.sbuf_tensor("g_sb", [P, NCHUNK, D], f32))
        diff_sb = ctx.enter_context(nc.sbuf_tensor("diff_sb", [P, 2, D], f32))
        sq_sb = ctx.enter_context(nc.sbuf_tensor("sq_sb", [P, D], f32))
        lbl_sb = ctx.enter_context(nc.sbuf_tensor("lbl_sb", [P, NCHUNK], mybir.dt.int32))
        acc_sb = ctx.enter_context(nc.sbuf_tensor("acc_sb", [P, NCHUNK], f32))

        block = ctx.enter_context(nc.Block())
        ls = ctx.enter_context(nc.semaphore("ls"))
        xs = [ctx.enter_context(nc.semaphore(f"xs{q}")) for q in range(NXD)]
        gs = [ctx.enter_context(nc.semaphore(f"gs{r}")) for r in range(NCHUNK)]
        vs = ctx.enter_context(nc.semaphore("vs"))
        ss = ctx.enter_context(nc.semaphore("ss"))
        os_ = ctx.enter_context(nc.semaphore("os"))

        CPX = NCHUNK // NXD  # chunks per x load

        @block.sync
        def _(sync):
            # labels first: the gather stream (Q7 descriptor emission) is the
            # critical path and only needs this tiny tile
            sync.dma_start(out=lbl_sb[:], in_=lbl_d[:]).then_inc(ls, 16)
            for q in range(NXD):
                sync.dma_start(
                    out=x_sb[:, q * CPX : (q + 1) * CPX, :],
                    in_=x_r[:, q * CPX : (q + 1) * CPX, :],
                ).then_inc(xs[q], 16)

        @block.gpsimd
        def _(gpsimd):
            gpsimd.wait_ge(ls, 16)
            for r in range(NCHUNK):
                gpsimd.indirect_dma_start(
                    out=g_sb[:, r, :],
                    out_offset=None,
                    in_=cen_d[:],
                    in_offset=bass.IndirectOffsetOnAxis(
                        ap=lbl_sb[:, r : r + 1], axis=0
                    ),
                ).then_inc(gs[r], 16)

        @block.vector
        def _(vector):
            for r in range(NCHUNK):
                vector.wait_ge(xs[r // CPX], 16)
                vector.wait_ge(gs[r], 16)
                if r >= 2:
                    vector.wait_ge(ss, r - 1)  # WAR: scalar done with diff slot
                vector.tensor_tensor(
                    out=diff_sb[:, r % 2, :],
                    in0=x_sb[:, r, :],
                    in1=g_sb[:, r, :],
                    op=mybir.AluOpType.subtract,
                ).then_inc(vs, 1)

        @block.scalar
        def _(scalar):
            for r in range(NCHUNK):
                scalar.wait_ge(vs, r + 1)
                scalar.activation(
                    out=sq_sb[:, :],
                    in_=diff_sb[:, r % 2, :],
                    func=mybir.ActivationFunctionType.Square,
                    accum_out=acc_sb[:, r : r + 1],
                ).then_inc(ss, 1)
                if r == NCHUNK - 3:
                    # early store of the first 14 columns hides most of the
                    # final DMA's completion receipt behind the last chunks
                    scalar.dma_start(
                        out=out_d[:, : NCHUNK - 2], in_=acc_sb[:, : NCHUNK - 2]
                    ).then_inc(os_, 16)
            scalar.dma_start(
                out=out_d[:, NCHUNK - 2 :], in_=acc_sb[:, NCHUNK - 2 :]
            ).then_inc(os_, 16)
            scalar.wait_ge(os_, 32)

    nc.finalize()
    return nc


def _get_nc():
    if "nc" not in _NC_CACHE:
        _NC_CACHE["nc"] = build_nc()
    return _NC_CACHE["nc"]


def kernel(x, labels, centers, _trace=False):
    x = np.asarray(x, dtype=np.float32)
    centers = np.asarray(centers, dtype=np.float32)
    labels_i = np.asarray(labels).astype(np.int32)

    in_maps = []
    for i in range(N_CORES):
        xs_ = np.ascontiguousarray(x[i * B_SHARD : (i + 1) * B_SHARD])
        ls_ = labels_i[i * B_SHARD : (i + 1) * B_SHARD]
        in_maps.append(
            {
                "x": xs_,
                # row 16p + r at [p, r]
                "labels": np.ascontiguousarray(ls_.reshape(P, NCHUNK)),
                "centers": centers,
            }
        )

    nc = _get_nc()
    res = run_bass_kernel_spmd(nc, in_maps, list(range(N_CORES)), trace=_trace)
    partials = np.stack([r["out"] for r in res.results])  # [8, 128, 16]
    total = np.sum(partials.astype(np.float64))
    total += B * (C - 1) * CLAMP_MIN
    loss = np.float32(total / B)
    if _trace:
        return np.asarray(loss), res
    return np.asarray(loss)



# revision 8
# speedup vs baseline: 1.5240x; 1.5240x over previous
"""CenterLoss kernel for Trainium2, data-parallel across 8 NeuronCores.

Math: the reference masks the full [B, C] squared-distance matrix with
one_hot(labels) and clamps to [1e-12, 1e12] before summing.  The mask keeps
only distmat[i, labels[i]]; every other entry becomes clip(0) = 1e-12.  The
kept entries are ~1024 (chi-square-like, 512 dof), so the clamp never binds
and the loss reduces to

    loss = ( sum_i ||x_i - c_{l_i}||^2 + B*(C-1)*1e-12 ) / B
         = ( sum_i |x_i|^2 + sum_c n_c |c_c|^2 - 2 sum_c S_c . c_c + ... ) / B

with S_c = sum of x_i over samples of class c and n_c the class counts.

The loss is permutation-invariant, so the host SORTS the batch by label
(pure data layout / sharding) and shards contiguously: each core's 2048
samples then span ~125 consecutive classes.  Each half (group) of 1024
samples spans <=128 classes, so per-class sums S come from EIGHT accumulated
128x128 one-hot matmuls per group on the TensorEngine — no indirect-DMA
gather at all.  Inputs are cast to bf16 (tolerance is 2e-2; bf16 error here
is ~1e-4), halving DMA traffic.

Per core (raw bass, chunk k = samples [128k, 128(k+1)), partition = sample
within chunk):
  sync   : input DMAs (labels/iota/centers-slice/counts first, then 8 x 256KB
           x pieces)
  vector : 16x one-hot tiles oh[p, c] = (lbl[p,k] == c) via tensor_scalar
           is_equal against an iota row; 8x fused square+rowsum of x chunks
           (tensor_tensor_reduce); 2x PSUM drain sum(S . cent) per group;
           n_c*|c|^2 term
  tensor : 16x matmul S_g += oh_k^T @ x_k  (bf16, FWL stationary)
  scalar : |cent|^2 rowsums, 8x square+rowsum of x chunks, output DMAs
Host sums the 8 x [128, 16+2G] partials in f64 and adds the clamp constant.
"""

import sys
from contextlib import ExitStack

import numpy as np
import ml_dtypes

try:
    import concourse.bass  # noqa: F401
except ImportError:
    sys.path.insert(0, "/opt/trn_rl_repo")

import concourse.mybir as mybir
from concourse.bacc import Bacc
from concourse.bass_utils import run_bass_kernel_spmd

B, C, D = 16384, 1000, 512
N_CORES = 8
B_SHARD = B // N_CORES  # 2048
P = 128
NCHUNK = B_SHARD // P  # 16 chunks, chunk k = samples [128k, 128(k+1))
NPIECE = 8  # x DMA pieces (2 chunks each)
CPP = NCHUNK // NPIECE
CLAMP_MIN = 1e-12

BF16 = ml_dtypes.bfloat16

_NC_CACHE = {}


def build_nc(G):
    """G = number of class-groups per core (each spans <=128 classes)."""
    CPG = NCHUNK // G  # chunks per group
    NOUT = NPIECE + 2 * G
    nc = Bacc(detect_race_conditions=False)
    f32 = mybir.dt.float32
    bf16 = mybir.dt.bfloat16

    x_d = nc.declare_dram_parameter("x", [B_SHARD, D], bf16, isOutput=False)
    lbl_d = nc.declare_dram_parameter("lbl", [P, NCHUNK], f32, isOutput=False)
    cen_d = nc.declare_dram_parameter("cent", [G, P, D], bf16, isOutput=False)
    cnt_d = nc.declare_dram_parameter("cnt", [P, G], f32, isOutput=False)
    iota_d = nc.declare_dram_parameter("iota", [P, P], bf16, isOutput=False)
    out_d = nc.declare_dram_parameter("out", [P, NOUT], f32, isOutput=True)

    x_r = x_d.rearrange("(k p) d -> p k d", p=P)  # chunk k, partition = sample
    cen_r = cen_d.rearrange("g m d -> m g d")

    with ExitStack() as ctx:
        x_sb = ctx.enter_context(nc.sbuf_tensor("x_sb", [P, NCHUNK, D], bf16))
        oh_sb = ctx.enter_context(nc.sbuf_tensor("oh_sb", [P, NCHUNK, P], bf16))
        cent_sb = ctx.enter_context(nc.sbuf_tensor("cent_sb", [P, G, D], bf16))
        lbl_sb = ctx.enter_context(nc.sbuf_tensor("lbl_sb", [P, NCHUNK], f32))
        cnt_sb = ctx.enter_context(nc.sbuf_tensor("cnt_sb", [P, G], f32))
        iota_sb = ctx.enter_context(nc.sbuf_tensor("iota_sb", [P, P], bf16))
        sqv_sb = ctx.enter_context(nc.sbuf_tensor("sqv_sb", [P, 2, D], bf16))
        sqa_sb = ctx.enter_context(nc.sbuf_tensor("sqa_sb", [P, 2, CPP, D], bf16))
        csq_sb = ctx.enter_context(nc.sbuf_tensor("csq_sb", [P, G], f32))
        acc_sb = ctx.enter_context(nc.sbuf_tensor("acc_sb", [P, NOUT], f32))
        s_ps = ctx.enter_context(nc.psum_tensor("s_ps", [P, G, D], f32))

        block = ctx.enter_context(nc.Block())
        ls = ctx.enter_context(nc.semaphore("ls"))
        is_ = ctx.enter_context(nc.semaphore("is_"))
        cs = ctx.enter_context(nc.semaphore("cs"))
        ns = ctx.enter_context(nc.semaphore("ns"))
        xs = [ctx.enter_context(nc.semaphore(f"xs{q}")) for q in range(NPIECE)]
        ohs = ctx.enter_context(nc.semaphore("ohs"))
        mms = ctx.enter_context(nc.semaphore("mms"))
        vr = ctx.enter_context(nc.semaphore("vr"))
        aq = ctx.enter_context(nc.semaphore("aq"))
        vd = ctx.enter_context(nc.semaphore("vd"))
        cq = ctx.enter_context(nc.semaphore("cq"))
        od = ctx.enter_context(nc.semaphore("od"))

        @block.sync
        def _(sync):
            sync.dma_start(out=lbl_sb[:], in_=lbl_d[:]).then_inc(ls, 16)
            sync.dma_start(out=iota_sb[:], in_=iota_d[:]).then_inc(is_, 16)
            sync.dma_start(out=cent_sb[:], in_=cen_r[:]).then_inc(cs, 16)
            sync.dma_start(out=cnt_sb[:], in_=cnt_d[:]).then_inc(ns, 16)
            for q in range(NPIECE):
                sync.dma_start(
                    out=x_sb[:, q * CPP : (q + 1) * CPP, :],
                    in_=x_r[:, q * CPP : (q + 1) * CPP, :],
                ).then_inc(xs[q], 16)

        @block.vector
        def _(vector):
            vector.wait_ge(ls, 16)
            vector.wait_ge(is_, 16)
            for k in range(NCHUNK):
                vector.tensor_scalar(
                    out=oh_sb[:, k, :],
                    in0=iota_sb[:, :],
                    scalar1=lbl_sb[:, k : k + 1],
                    scalar2=None,
                    op0=mybir.AluOpType.is_equal,
                ).then_inc(ohs, 1)
            vector.wait_ge(cs, 16)
            for g in range(G):
                vector.wait_ge(mms, g + 1)
                vector.tensor_tensor(
                    out=sqv_sb[:, g % 2, :],
                    in0=s_ps[:, g, :],
                    in1=cent_sb[:, g, :],
                    op=mybir.AluOpType.mult,
                ).then_inc(vr, 1)
                # same-engine RAW needs an explicit semaphore on this HW
                vector.wait_ge(vr, g + 1)
                vector.tensor_reduce(
                    out=acc_sb[:, NPIECE + g : NPIECE + g + 1],
                    in_=sqv_sb[:, g % 2, :],
                    axis=mybir.AxisListType.X,
                    op=mybir.AluOpType.add,
                ).then_inc(vd, 1)
            vector.wait_ge(cq, G)
            vector.wait_ge(ns, 16)
            vector.tensor_tensor(
                out=acc_sb[:, NPIECE + G : NPIECE + 2 * G],
                in0=csq_sb[:, :],
                in1=cnt_sb[:, :],
                op=mybir.AluOpType.mult,
            ).then_inc(vd, 1)

        @block.tensor
        def _(tensor):
            for k in range(NCHUNK):
                tensor.wait_ge(ohs, k + 1)
                tensor.wait_ge(xs[k // CPP], 16)
                mm = tensor.matmul(
                    s_ps[:, k // CPG, :],
                    oh_sb[:, k, :],
                    x_sb[:, k, :],
                    start=(k % CPG == 0),
                    stop=(k % CPG == CPG - 1),
                )
                if k % CPG == CPG - 1:
                    mm.then_inc(mms, 1)

        @block.scalar
        def _(scalar):
            scalar.wait_ge(cs, 16)
            for g in range(G):
                scalar.activation(
                    out=sqa_sb[:, g % 2, 0, :],
                    in_=cent_sb[:, g, :],
                    func=mybir.ActivationFunctionType.Square,
                    accum_out=csq_sb[:, g : g + 1],
                ).then_inc(cq, 1)
            for q in range(NPIECE):
                scalar.wait_ge(xs[q], 16)
                scalar.activation(
                    out=sqa_sb[:, q % 2, :, :],
                    in_=x_sb[:, q * CPP : (q + 1) * CPP, :],
                    func=mybir.ActivationFunctionType.Square,
                    accum_out=acc_sb[:, q : q + 1],
                ).then_inc(aq, 1)
            # engine-issued DMA does not serialize with the engine's own
            # in-flight datapath: gate on our own accum retirements
            scalar.wait_ge(aq, NPIECE)
            scalar.dma_start(out=out_d[:, :NPIECE], in_=acc_sb[:, :NPIECE]).then_inc(
                od, 16
            )
            scalar.wait_ge(vd, G + 1)
            scalar.dma_start(out=out_d[:, NPIECE:], in_=acc_sb[:, NPIECE:]).then_inc(
                od, 16
            )
            scalar.wait_ge(od, 32)

    nc.finalize()
    return nc


def _get_nc(G):
    if G not in _NC_CACHE:
        _NC_CACHE[G] = build_nc(G)
    return _NC_CACHE[G]


def _shard_inputs(x, labels, centers, G):
    """Sort by label, shard, build per-core input maps.  Returns None if some
    group spans more than 128 classes (caller retries with larger G)."""
    perm = np.argsort(labels, kind="stable")
    lab_s = labels[perm]
    seg = B_SHARD // G  # samples per group
    iota = np.ascontiguousarray(
        np.broadcast_to(np.arange(P, dtype=BF16), (P, P))
    )
    in_maps = []
    for i in range(N_CORES):
        sl = slice(i * B_SHARD, (i + 1) * B_SHARD)
        idx = perm[sl]
        lab = lab_s[sl]
        lo = np.empty(G, dtype=np.int64)
        for g in range(G):
            s0, s1 = g * seg, (g + 1) * seg
            lo[g] = lab[s0]
            if lab[s1 - 1] - lab[s0] + 1 > P:
                return None
        xs_ = np.ascontiguousarray(x[idx]).astype(BF16)
        cent = np.zeros((G, P, D), dtype=BF16)
        for g in range(G):
            take = min(P, C - lo[g])
            cent[g, :take] = centers[lo[g] : lo[g] + take]
        reb = lab - np.repeat(lo, seg)  # rebased labels in [0, 128)
        lblf = np.ascontiguousarray(
            reb.reshape(NCHUNK, P).T.astype(np.float32)
        )  # [p, k]
        cnt = np.zeros((P, G), dtype=np.float32)
        for g in range(G):
            bc = np.bincount(reb[g * seg : (g + 1) * seg], minlength=P)
            cnt[:, g] = bc.astype(np.float32)
        in_maps.append(
            {
                "x": xs_,
                "lbl": lblf,
                "cent": cent,
                "cnt": cnt,
                "iota": iota,
            }
        )
    return in_maps


def kernel(x, labels, centers, _trace=False):
    x = np.asarray(x, dtype=np.float32)
    centers = np.asarray(centers, dtype=np.float32)
    labels_i = np.asarray(labels).astype(np.int64)

    for G in (2, 4, 8):
        in_maps = _shard_inputs(x, labels_i, centers, G)
        if in_maps is not None:
            break
    else:
        raise ValueError("label distribution too skewed for G<=8 grouping")

    nc = _get_nc(G)
    res = run_bass_kernel_spmd(nc, in_maps, list(range(N_CORES)), trace=_trace)
    parts = np.stack([r["out"] for r in res.results]).astype(np.float64)
    xsq = parts[:, :, :NPIECE].sum()
    cross = parts[:, :, NPIECE : NPIECE + G].sum()
    ncsq = parts[:, :, NPIECE + G :].sum()
    total = xsq + ncsq - 2.0 * cross
    total += B * (C - 1) * CLAMP_MIN
    loss = np.float32(total / B)
    if _trace:
        return np.asarray(loss), res
    return np.asarray(loss)
